# revision 9
# baseline (speedup 1.0000x reference)
"""Trainium2 Bass kernel for nn_ClassifierLSTM (2-layer masked LSTM classifier).

Tensor-parallel over the gate dimension across 8 NeuronCores: each core owns
128 hidden units (512 gate columns arranged [i|f|o|g]) of both LSTM layers.

Design (vs. the serialized interleave-by-1 baseline):
 - 2x PE column tiling: z1 accumulates in psum partitions [64p, 64p+64)
   (p = step parity) while layer-2's recurrent (r1) matmuls accumulate in
   the opposite half of the xz2 batch tile; the two matmul streams are
   emitted interleaved so the two 128x64 array tiles stream concurrently
   (matmuls issue strictly in program order).
 - layer 2 lags layer 1 by 3 steps; its input projection xz2 = h1 @ k1 + b1
   (+ mask pattern) is computed in M=128 batches of TWO timesteps at full
   array width into a psum ring the r1 recurrence accumulates into. The
   batch matmuls are emitted at the iteration tail so they fill the
   collective-wait window instead of delaying the h-exchange fires.
 - two per-layer AllGathers per step (h1 right after cell1, h2 after
   cell2), each with most of an iteration of independent PE work to hide
   behind; premult rows for two steps are fetched by one indirect DMA via
   a pair-major token index, and the per-step inject selects the pair half
   with a top/bottom identity stationary (stays column-tile paired).
 - Keras mask_zero folded into the gate inputs: premult row 0 (the masked
   token's row) carries a [-30|+30|0|0] i/f pattern; xz2 batches get the
   pattern AND b1 via one K=2 matmul ([mask row; ones row] stationary).
   Cells then need only 3 activation instructions each (sigmoid over
   [i|f|o], tanh(g), tanh(c)).
 - gate tensors bf16 where safe (sigmoid outputs stay f32 to protect the
   512-step cell-state recursion) for 2x DVE throughput.

Caution: matmul stationary reads at free-dim offsets beyond ~32KB into a
partition silently return garbage — keep per-step lookup tables split into
small tiles (m30C is 4x 16KB).
"""
import os
import sys

sys.path.insert(0, "/opt/trn_rl_repo")

import numpy as np
import ml_dtypes

import concourse.bass as bass
import concourse.mybir as mybir
import concourse.tile as tile
from concourse import bacc
from concourse.bass_utils import run_bass_kernel_spmd

F32 = mybir.dt.float32
BF16 = mybir.dt.bfloat16
I32 = mybir.dt.int32
AF = mybir.ActivationFunctionType
OP = mybir.AluOpType

VOCAB, EMB, HID, OUTD = 32000, 512, 1024, 3
B, T = 64, 512
NC = 8
SH = 512  # gate columns per core (= 4 * 128 hidden units)
HSH = 128  # hidden units per core
KC = HID // 128  # 8 K-chunks for recurrent / layer-2 matmuls
KC0 = EMB // 128  # 4 K-chunks for the premult matmul
VT = VOCAB // 128  # 250 vocab tiles
RB = 4  # bounce-buffer ring depth
LAG = 3  # layer-2 step lag behind layer 1


def build(t_steps=T):
    assert t_steps % 2 == 0, "k1 batching assumes an even number of steps"
    nc = bacc.Bacc("TRN2", target_bir_lowering=False, debug=False, num_devices=NC)

    # ---- I/O ----
    tok = nc.dram_tensor("tok", [B, T], I32, kind="ExternalInput")
    embT = nc.dram_tensor("embT", [EMB, VOCAB], BF16, kind="ExternalInput")
    k0s = nc.dram_tensor("k0s", [EMB, SH], BF16, kind="ExternalInput")
    r0s = nc.dram_tensor("r0s", [HID, SH], BF16, kind="ExternalInput")
    k1s = nc.dram_tensor("k1s", [HID, SH], BF16, kind="ExternalInput")
    r1s = nc.dram_tensor("r1s", [HID, SH], BF16, kind="ExternalInput")
    b0s = nc.dram_tensor("b0s", [1, SH], BF16, kind="ExternalInput")
    patb = nc.dram_tensor("patb", [2, SH], BF16, kind="ExternalInput")
    wout = nc.dram_tensor("wout", [HID, OUTD], BF16, kind="ExternalInput")
    bout = nc.dram_tensor("bout", [1, OUTD], BF16, kind="ExternalInput")
    eye128 = nc.dram_tensor("eye128", [128, B], BF16, kind="ExternalInput")
    eyebot = nc.dram_tensor("eyebot", [128, B], BF16, kind="ExternalInput")
    tokP = nc.dram_tensor("tokP", [128, T // 2], I32, kind="ExternalInput")
    eye64 = nc.dram_tensor("eye64", [B, B], BF16, kind="ExternalInput")
    m30C = nc.dram_tensor("m30C", [2, T // 2, 2 * B], BF16, kind="ExternalInput")
    mflt = nc.dram_tensor("mflt", [B, T], F32, kind="ExternalInput")
    pat1 = nc.dram_tensor("pat1", [1, SH], BF16, kind="ExternalInput")
    patf = nc.dram_tensor("patf", [1, SH], F32, kind="ExternalInput")
    out = nc.dram_tensor("out", [B, OUTD], F32, kind="ExternalOutput")

    # ---- internal DRAM ----
    premult = nc.dram_tensor("premult", [VOCAB, SH], BF16)
    # combined h1|h2 exchange: one AllGather per step of [HSH, 2B]
    # (h1T in cols 0:B, h2T in cols B:2B)
    agin = [nc.dram_tensor(f"agin_{k}", [HSH, 2 * B], BF16) for k in range(RB)]
    agout = [
        nc.dram_tensor(f"agout_{k}", [NC * HSH, 2 * B], BF16, addr_space="Shared")
        for k in range(RB)
    ]


    with tile.TileContext(nc) as tc:
        with (
            tc.tile_pool(name="persist", bufs=1) as pp,
            tc.tile_pool(name="wpool", bufs=1) as wp,
        ):
            # --- resident tiles ---
            tokS = pp.tile([B, T], I32)
            nc.sync.dma_start(tokS[:], tok[:])
            mS = pp.tile([B, T], F32)
            nc.sync.dma_start(mS[:], mflt[:])
            # mask+ones stationary pairs, in 64-pair tiles so
            # stationary-AP free offsets stay small (<16KB)
            NP4 = T // 8
            m30Cs = []
            for k in range(4):
                mt = pp.tile([2, NP4, 2 * B], BF16, name=f"m30C_{k}")
                nc.sync.dma_start(mt[:], m30C[:, k * NP4 : (k + 1) * NP4, :])
                m30Cs.append(mt)
            tokPS = pp.tile([128, T // 2], I32)
            nc.sync.dma_start(tokPS[:], tokP[:])

            eye128S = pp.tile([128, B], BF16)
            nc.sync.dma_start(eye128S[:], eye128[:])
            eyebotS = pp.tile([128, B], BF16)
            nc.sync.dma_start(eyebotS[:], eyebot[:])
            eye64S = pp.tile([B, B], BF16)
            nc.sync.dma_start(eye64S[:], eye64[:])
            patbS = pp.tile([2, SH], BF16)
            nc.sync.dma_start(patbS[:], patb[:])
            pat1S = pp.tile([1, SH], BF16)
            nc.sync.dma_start(pat1S[:], pat1[:])
            patfS = pp.tile([1, SH], F32)
            nc.sync.dma_start(patfS[:], patf[:])
            boutS = pp.tile([1, OUTD], BF16)
            nc.sync.dma_start(boutS[:], bout[:])
            b0S = pp.tile([1, SH], BF16)
            nc.sync.dma_start(b0S[:], b0s[:])

            r0S = wp.tile([128, KC, SH], BF16)
            nc.sync.dma_start(r0S[:], r0s[:].rearrange("(c p) n -> p c n", p=128))
            k1S = wp.tile([128, KC, SH], BF16)
            nc.sync.dma_start(k1S[:], k1s[:].rearrange("(c p) n -> p c n", p=128))
            r1S = wp.tile([128, KC, SH], BF16)
            nc.sync.dma_start(r1S[:], r1s[:].rearrange("(c p) n -> p c n", p=128))
            k0S = wp.tile([128, KC0, SH], BF16)
            nc.sync.dma_start(k0S[:], k0s[:].rearrange("(c p) n -> p c n", p=128))
            woutS = wp.tile([128, KC, OUTD], BF16)
            nc.sync.dma_start(woutS[:], wout[:].rearrange("(c p) n -> p c n", p=128))

            # gathered hT ring tiles (persistent; halves/slots written per step)
            hr1 = [pp.tile([128, KC, 128], BF16, name=f"hr1_{i}") for i in range(3)]
            hr2 = [pp.tile([128, KC, B], BF16, name=f"hr2_{i}") for i in range(3)]
            # premult-row pair-gather ring (rows 0:64 = even step,
            # 64:128 = odd step of the pair)
            g4 = [pp.tile([128, SH], BF16, name=f"g4_{i}") for i in range(RB)]

            # --- phase 1: premult = emb @ k0_shard + b0_shard (+ row-0 mask
            # pattern: token 0 is the masked token) ---
            onesS = pp.tile([1, 128], BF16)
            nc.vector.memset(onesS[:], 1.0)
            with (
                tc.tile_pool(name="pm_sb", bufs=4) as pmsb,
                tc.tile_pool(name="pm_ps", bufs=2, space="PSUM") as pmps,
            ):
                for v in range(VT):
                    et = pmsb.tile([128, KC0, 128], BF16, tag="embtile")
                    nc.sync.dma_start(
                        et[:], embT[:, v * 128 : (v + 1) * 128].rearrange(
                            "(c p) n -> p c n", p=128
                        )
                    )
                    ps = pmps.tile([128, SH], F32)
                    nc.tensor.matmul(
                        ps[:], onesS[:1, :], b0S[:1, :], start=True, stop=False
                    )
                    for c in range(KC0):
                        nc.tensor.matmul(
                            ps[:], et[:, c, :], k0S[:, c, :],
                            start=False, stop=(c == KC0 - 1),
                        )
                    pv = pmsb.tile([128, SH], BF16, tag="pmtile")
                    nc.vector.tensor_copy(pv[:], ps[:])
                    if v == 0:
                        # masked-token row: fold the i/f forcing pattern in
                        nc.vector.tensor_tensor(
                            pv[0:1, :], ps[0:1, :], patfS[0:1, :], OP.add
                        )
                    nc.sync.dma_start(premult[v * 128 : (v + 1) * 128, :], pv[:])

            # --- phase 2: recurrences (layer 2 lags by LAG steps) ---
            with (
                tc.tile_pool(name="state", bufs=2) as st,
                tc.tile_pool(name="gates", bufs=3) as gt,
                tc.tile_pool(name="zps", bufs=2, space="PSUM") as zps,
                tc.tile_pool(name="xzps", bufs=3, space="PSUM") as xzps,
                tc.tile_pool(name="trps", bufs=1, space="PSUM") as trps,
                tc.tile_pool(name="wps", bufs=1, space="PSUM") as wps,
            ):
                c1 = st.tile([B, HSH], F32, tag="c1")
                h1 = st.tile([B, HSH], BF16, tag="h1")
                c2 = st.tile([B, HSH], F32, tag="c2")
                h2 = st.tile([B, HSH], BF16, tag="h2")
                for tl in (c1, h1, c2, h2):
                    nc.vector.memset(tl[:], 0.0)

                xz_tiles = {}  # pair index (s//2) -> psum tile

                def cell(zh, li, t_idx, c_old, h_old, trs_dst):
                    """Gates+cell+mask for one layer step.

                    zh: [64, 512] psum slice holding z (+/-30 i/f pattern
                    already folded in for masked steps). Returns
                    (c_new, h_new, trs) where trs is h_new^T in SBUF.
                    """
                    mcol = mS[:, t_idx : t_idx + 1]
                    sg = gt.tile([B, 384], F32, tag=f"sg{li}")
                    nc.scalar.activation(sg[:], zh[:, 0:384], AF.Sigmoid)
                    gg = gt.tile([B, HSH], BF16, tag=f"gg{li}")
                    nc.scalar.activation(gg[:], zh[:, 384:512], AF.Tanh)
                    u = gt.tile([B, HSH], BF16, tag=f"u{li}")
                    nc.vector.tensor_tensor(u[:], sg[:, 0:128], gg[:], OP.mult)
                    v = gt.tile([B, HSH], F32, tag=f"v{li}")
                    # off the Pool engine: gpsimd's queue must stay clear so
                    # each iteration's collective doorbell issues promptly
                    nc.vector.tensor_tensor(v[:], sg[:, 128:256], c_old[:], OP.mult)
                    c_new = st.tile([B, HSH], F32, tag=f"c{li}")
                    nc.vector.tensor_tensor(c_new[:], u[:], v[:], OP.add)
                    th = gt.tile([B, HSH], BF16, tag=f"th{li}")
                    nc.scalar.activation(th[:], c_new[:], AF.Tanh)
                    hn = gt.tile([B, HSH], BF16, tag=f"hn{li}")
                    nc.vector.tensor_tensor(hn[:], sg[:, 256:384], th[:], OP.mult)
                    dh = gt.tile([B, HSH], BF16, tag=f"dh{li}")
                    nc.vector.tensor_tensor(dh[:], hn[:], h_old[:], OP.subtract)
                    h_new = st.tile([B, HSH], BF16, tag=f"h{li}")
                    nc.vector.scalar_tensor_tensor(
                        h_new[:], dh[:], mcol, h_old[:], OP.mult, OP.add
                    )
                    trp = trps.tile([HSH, B], BF16, tag=f"tr{li}")
                    nc.tensor.transpose(trp[:], h_new[:], eye64S[:])
                    nc.vector.tensor_copy(trs_dst, trp[:])
                    return c_new, h_new

                def hT1_slice(step, c):
                    j = (step // 2) % 3
                    off = 64 * (step % 2)
                    return hr1[j][:, c, off : off + 64]

                for t in range(t_steps + LAG):
                    slot = t % RB
                    p = t % 2
                    has_z1 = t < t_steps
                    has_l2 = t >= LAG
                    s = t - LAG
                    sp = s % 2

                    # ---- gather premult rows two step-pairs ahead so the
                    # gather is immune to GpSimd issue-queue jitter (ring of
                    # 4 holds pairs P..P+3) ----
                    if t == 0:
                        gps = [0, 1]
                    elif t % 2 == 1:
                        gps = [(t + 3) // 2]
                    else:
                        gps = []
                    for gpair in gps:
                        if gpair < t_steps // 2:
                            nc.gpsimd.indirect_dma_start(
                                out=g4[gpair % RB][:],
                                out_offset=None,
                                in_=premult[:],
                                in_offset=bass.IndirectOffsetOnAxis(
                                    ap=tokPS[:, gpair : gpair + 1], axis=0
                                ),
                            )
                    if has_z1:
                        gtile = g4[(t // 2) % RB]
                        zt = zps.tile([128, SH], F32, tag="z1")
                        z1 = zt[64 * p : 64 * p + 64, :]
                    if has_l2:
                        xt2 = xz_tiles[s // 2]
                        z2 = xt2[64 * sp : 64 * sp + 64, :]

                    # ---- z1 (col tile p) and r1 (col tile 1-p) matmuls,
                    # emitted interleaved so the two 128x64 array tiles
                    # stream concurrently (MMs issue in program order) ----
                    if has_z1:
                        eyeh = eye128S if p == 0 else eyebotS
                        nc.tensor.matmul(
                            z1, eyeh[:], gtile[:], start=True, stop=(t == 0)
                        )
                    for c in range(KC):
                        if has_z1 and t > 0:
                            nc.tensor.matmul(
                                z1, hT1_slice(t - 1, c), r0S[:, c, :],
                                start=False, stop=(c == KC - 1),
                            )
                        if has_l2 and s > 0:
                            nc.tensor.matmul(
                                z2, hr2[(t - 1) % 3][:, c, :], r1S[:, c, :],
                                start=False, stop=(c == KC - 1),
                                skip_group_check=True,
                            )

                    # shared staging tile: h1T | h2T side by side, ONE DMA out
                    trsP = gt.tile([HSH, 2 * B], BF16, tag="trsP")
                    if has_z1:
                        # ---- layer-1 cell ----
                        c1, h1 = cell(z1, 1, t, c1, h1, trsP[:, 0:B])

                    if has_l2:
                        # ---- layer-2 cell for step s ----
                        c2, h2 = cell(z2, 2, s, c2, h2, trsP[:, B : 2 * B])
                        if sp == 1:
                            xz_tiles.pop(s // 2, None)

                    nc.sync.dma_start(agin[slot][:], trsP[:])

                    # ---- ONE combined h1|h2 AllGather per iteration ----
                    nc.gpsimd.collective_compute(
                        "AllGather",
                        OP.bypass,
                        replica_groups=[list(range(NC))],
                        ins=[agin[slot][:].opt()],
                        outs=[agout[slot][:].opt()],
                    )
                    if has_z1:
                        nc.sync.dma_start(
                            hr1[(t // 2) % 3][:, :, 64 * p : 64 * p + 64],
                            agout[slot][:, 0:B].rearrange(
                                "(c p) n -> p c n", p=128
                            ),
                        )
                    if has_l2:
                        nc.sync.dma_start(
                            hr2[t % 3][:],
                            agout[slot][:, B : 2 * B].rearrange(
                                "(c p) n -> p c n", p=128
                            ),
                        )

                    if t >= 2 and t % 2 == 0 and t <= t_steps:
                        # ---- batched layer-2 input projection for steps
                        # (t-2, t-1) at full array width ----
                        pair = (t - 2) // 2
                        xt = xzps.tile([128, SH], F32, tag="xz2")
                        xz_tiles[pair] = xt
                        # K=2 stationary [30*(1-mask) pair row; ones row]
                        # with rhs [i/f pattern; b1]: adds mask forcing AND
                        # the layer-2 bias in one matmul
                        mrow = m30Cs[pair // NP4][0:2, pair % NP4, :]
                        nc.tensor.matmul(
                            xt[:], mrow, patbS[0:2, :],
                            start=True, stop=False, skip_group_check=True,
                        )
                        for c in range(KC):
                            nc.tensor.matmul(
                                xt[:], hr1[pair % 3][:, c, :], k1S[:, c, :],
                                start=False, stop=(c == KC - 1),
                                skip_group_check=True,
                            )

                # ---- output: logits = h2_final @ Wout + bout ----
                tf = (t_steps + LAG - 1) % 3
                ops = wps.tile([B, OUTD], F32, tag="ops")
                nc.tensor.matmul(
                    ops[:], onesS[:1, :B], boutS[:1, :], start=True, stop=False
                )
                for c in range(KC):
                    nc.tensor.matmul(
                        ops[:], hr2[tf][:, c, :], woutS[:, c, :],
                        start=False, stop=(c == KC - 1),
                    )
                logits = gt.tile([B, OUTD], F32, tag="logits")
                nc.vector.tensor_copy(logits[:], ops[:])
                nc.sync.dma_start(out[:], logits[:])

    nc.finalize()
    return nc


_CACHE = {}


def _get_nc(t_steps):
    if t_steps not in _CACHE:
        _CACHE[t_steps] = build(t_steps)
    return _CACHE[t_steps]


def _prep_in_maps(inputs, emb, k0, r0, b0, k1, r1, b1, Wout, bout):
    bf = ml_dtypes.bfloat16
    embT = np.ascontiguousarray(emb.T).astype(bf)
    tok = np.ascontiguousarray(inputs.astype(np.int32))
    t_steps = int(os.environ.get("LSTM_T", T))

    eye64 = np.eye(B, dtype=bf)
    eye128 = np.zeros((128, B), dtype=bf)
    eye128[:B] = np.eye(B)

    m = (tok != 0).astype(np.float32)  # [B, T]
    m30 = 30.0 * (1.0 - m)  # [B, T]
    # step-pair mask rows + ones rows: partition 0 = [m30[:,2P] | m30[:,2P+1]],
    # partition 1 = ones (carries b1 through the K=2 pattern matmul)
    m30C = np.stack(
        [m30.T.reshape(T // 2, 2 * B), np.ones((T // 2, 2 * B), np.float32)]
    ).astype(bf)
    # pair-major token indices: rows 0:64 = step 2P, rows 64:128 = step 2P+1
    tokP = np.ascontiguousarray(
        np.concatenate([tok[:, 0::2], tok[:, 1::2]], axis=0)
    ).astype(np.int32)
    eyebot = np.zeros((128, B), dtype=bf)
    eyebot[B:] = np.eye(B)

    pat1 = np.zeros((1, SH), dtype=bf)
    pat1[0, 0:128] = -1.0
    pat1[0, 128:256] = 1.0
    patf = np.zeros((1, SH), dtype=np.float32)
    patf[0, 0:128] = -30.0
    patf[0, 128:256] = 30.0

    in_maps = []
    for c in range(NC):
        hc = slice(c * HSH, (c + 1) * HSH)
        # per-core gate-column permutation: [i | f | o | g] blocks
        cols = np.concatenate(
            [
                np.arange(0 * HID, 1 * HID)[hc],  # i
                np.arange(1 * HID, 2 * HID)[hc],  # f
                np.arange(3 * HID, 4 * HID)[hc],  # o
                np.arange(2 * HID, 3 * HID)[hc],  # g
            ]
        )
        b1c = np.ascontiguousarray(b1[cols]).astype(np.float32)
        patb = np.stack([pat1[0].astype(np.float32), b1c]).astype(bf)
        in_maps.append(
            {
                "tok": tok,
                "embT": embT,
                "k0s": np.ascontiguousarray(k0[:, cols]).astype(bf),
                "r0s": np.ascontiguousarray(r0[:, cols]).astype(bf),
                "k1s": np.ascontiguousarray(k1[:, cols]).astype(bf),
                "r1s": np.ascontiguousarray(r1[:, cols]).astype(bf),
                "b0s": np.ascontiguousarray(b0[cols])[None, :].astype(bf),
                "patb": patb,
                "wout": np.ascontiguousarray(Wout).astype(bf),
                "bout": np.ascontiguousarray(bout)[None, :].astype(bf),
                "eye128": eye128,
                "eyebot": eyebot,
                "tokP": tokP,
                "eye64": eye64,
                "m30C": m30C,
                "mflt": m,

                "pat1": pat1,
                "patf": patf,
            }
        )
    return in_maps


def kernel(inputs, emb, k0, r0, b0, k1, r1, b1, Wout, bout, _trace=False):
    t_steps = int(os.environ.get("LSTM_T", T))
    nc = _get_nc(t_steps)
    in_maps = _prep_in_maps(
        np.asarray(inputs), np.asarray(emb), np.asarray(k0), np.asarray(r0),
        np.asarray(b0), np.asarray(k1), np.asarray(r1), np.asarray(b1),
        np.asarray(Wout), np.asarray(bout),
    )
    res = run_bass_kernel_spmd(
        nc, in_maps, core_ids=list(range(NC)), trace=_trace
    )
    kernel.last_result = res
    return res.results[0]["out"].astype(np.float32)



# revision 12
# speedup vs baseline: 1.0098x; 1.0098x over previous
"""Trainium2 Bass kernel for nn_ClassifierLSTM (2-layer masked LSTM classifier).

Tensor-parallel over the gate dimension across 8 NeuronCores: each core owns
128 hidden units (512 gate columns arranged [i|f|o|g]) of both LSTM layers.

Design (vs. the serialized interleave-by-1 baseline):
 - 2x PE column tiling: z1 accumulates in psum partitions [64p, 64p+64)
   (p = step parity) while layer-2's recurrent (r1) matmuls accumulate in
   the opposite half of the xz2 batch tile; the two matmul streams are
   emitted interleaved so the two 128x64 array tiles stream concurrently
   (matmuls issue strictly in program order).
 - layer 2 lags layer 1 by 3 steps; its input projection xz2 = h1 @ k1 + b1
   (+ mask pattern) is computed in M=128 batches of TWO timesteps at full
   array width into a psum ring the r1 recurrence accumulates into. The
   batch matmuls are emitted at the iteration tail so they fill the
   collective-wait window instead of delaying the h-exchange fires.
 - ONE combined AllGather per step: h1T(t) and h2T(t-LAG) are staged side
   by side in a single SBUF tile [HSH, 2B] by the two cells' transpose
   copies, written to DRAM with a single DMA (one writer -> the collective
   reliably waits on it; two half-writes raced under Tile's per-tensor
   last-writer DRAM tracking), gathered once, and split back into the hr1
   / hr2 rings by two column-sliced DMAs. Halves the per-step ncfw
   collective floor (~10us -> ~7.5us busy, Comms occupancy 62% -> 30%).
   Premult rows for two steps are fetched by one indirect DMA via
   a pair-major token index, and the per-step inject selects the pair half
   with a top/bottom identity stationary (stays column-tile paired).
 - Keras mask_zero folded into the gate inputs: premult row 0 (the masked
   token's row) carries a [-30|+30|0|0] i/f pattern; xz2 batches get the
   pattern AND b1 via one K=2 matmul ([mask row; ones row] stationary).
   Cells then need only 3 activation instructions each (sigmoid over
   [i|f|o], tanh(g), tanh(c)).
 - gate tensors bf16 where safe (sigmoid outputs stay f32 to protect the
   512-step cell-state recursion) for 2x DVE throughput.

Caution: matmul stationary reads at free-dim offsets beyond ~32KB into a
partition silently return garbage — keep per-step lookup tables split into
small tiles (m30C is 4x 16KB).
"""
import os
import sys

sys.path.insert(0, "/opt/trn_rl_repo")

import numpy as np
import ml_dtypes

import concourse.bass as bass
import concourse.mybir as mybir
import concourse.tile as tile
from concourse import bacc
from concourse.bass_utils import run_bass_kernel_spmd

F32 = mybir.dt.float32
BF16 = mybir.dt.bfloat16
I32 = mybir.dt.int32
AF = mybir.ActivationFunctionType
OP = mybir.AluOpType

VOCAB, EMB, HID, OUTD = 32000, 512, 1024, 3
B, T = 64, 512
NC = 8
SH = 512  # gate columns per core (= 4 * 128 hidden units)
HSH = 128  # hidden units per core
KC = HID // 128  # 8 K-chunks for recurrent / layer-2 matmuls
KC0 = EMB // 128  # 4 K-chunks for the premult matmul
VT = VOCAB // 128  # 250 vocab tiles
RB = 4  # bounce-buffer ring depth
LAG = 3  # layer-2 step lag behind layer 1


def build(t_steps=T):
    assert t_steps % 2 == 0, "k1 batching assumes an even number of steps"
    nc = bacc.Bacc("TRN2", target_bir_lowering=False, debug=False, num_devices=NC)

    # ---- I/O ----
    tok = nc.dram_tensor("tok", [B, T], I32, kind="ExternalInput")
    embT = nc.dram_tensor("embT", [EMB, VOCAB], BF16, kind="ExternalInput")
    k0s = nc.dram_tensor("k0s", [EMB, SH], BF16, kind="ExternalInput")
    r0s = nc.dram_tensor("r0s", [HID, SH], BF16, kind="ExternalInput")
    k1s = nc.dram_tensor("k1s", [HID, SH], BF16, kind="ExternalInput")
    r1s = nc.dram_tensor("r1s", [HID, SH], BF16, kind="ExternalInput")
    b0s = nc.dram_tensor("b0s", [1, SH], BF16, kind="ExternalInput")
    patb = nc.dram_tensor("patb", [2, SH], BF16, kind="ExternalInput")
    wout = nc.dram_tensor("wout", [HID, OUTD], BF16, kind="ExternalInput")
    bout = nc.dram_tensor("bout", [1, OUTD], BF16, kind="ExternalInput")
    eye128 = nc.dram_tensor("eye128", [128, B], BF16, kind="ExternalInput")
    eyebot = nc.dram_tensor("eyebot", [128, B], BF16, kind="ExternalInput")
    tokP = nc.dram_tensor("tokP", [128, T // 2], I32, kind="ExternalInput")
    eye64 = nc.dram_tensor("eye64", [B, B], BF16, kind="ExternalInput")
    m30C = nc.dram_tensor("m30C", [2, T // 2, 2 * B], BF16, kind="ExternalInput")
    mflt = nc.dram_tensor("mflt", [B, T], F32, kind="ExternalInput")
    pat1 = nc.dram_tensor("pat1", [1, SH], BF16, kind="ExternalInput")
    patf = nc.dram_tensor("patf", [1, SH], F32, kind="ExternalInput")
    out = nc.dram_tensor("out", [B, OUTD], F32, kind="ExternalOutput")

    # ---- internal DRAM ----
    premult = nc.dram_tensor("premult", [VOCAB, SH], BF16)
    # combined h1|h2 exchange: one AllGather per step of [HSH, 2B]
    # (h1T in cols 0:B, h2T in cols B:2B)
    agin = [nc.dram_tensor(f"agin_{k}", [HSH, 2 * B], BF16) for k in range(RB)]
    agout = [
        nc.dram_tensor(f"agout_{k}", [NC * HSH, 2 * B], BF16, addr_space="Shared")
        for k in range(RB)
    ]


    with tile.TileContext(nc) as tc:
        with (
            tc.tile_pool(name="persist", bufs=1) as pp,
            tc.tile_pool(name="wpool", bufs=1) as wp,
        ):
            # --- resident tiles ---
            tokS = pp.tile([B, T], I32)
            nc.sync.dma_start(tokS[:], tok[:])
            mS = pp.tile([B, T], F32)
            nc.sync.dma_start(mS[:], mflt[:])
            # mask+ones stationary pairs, in 64-pair tiles so
            # stationary-AP free offsets stay small (<16KB)
            NP4 = T // 8
            m30Cs = []
            for k in range(4):
                mt = pp.tile([2, NP4, 2 * B], BF16, name=f"m30C_{k}")
                nc.sync.dma_start(mt[:], m30C[:, k * NP4 : (k + 1) * NP4, :])
                m30Cs.append(mt)
            tokPS = pp.tile([128, T // 2], I32)
            nc.sync.dma_start(tokPS[:], tokP[:])

            eye128S = pp.tile([128, B], BF16)
            nc.sync.dma_start(eye128S[:], eye128[:])
            eyebotS = pp.tile([128, B], BF16)
            nc.sync.dma_start(eyebotS[:], eyebot[:])
            eye64S = pp.tile([B, B], BF16)
            nc.sync.dma_start(eye64S[:], eye64[:])
            patbS = pp.tile([2, SH], BF16)
            nc.sync.dma_start(patbS[:], patb[:])
            pat1S = pp.tile([1, SH], BF16)
            nc.sync.dma_start(pat1S[:], pat1[:])
            patfS = pp.tile([1, SH], F32)
            nc.sync.dma_start(patfS[:], patf[:])
            boutS = pp.tile([1, OUTD], BF16)
            nc.sync.dma_start(boutS[:], bout[:])
            b0S = pp.tile([1, SH], BF16)
            nc.sync.dma_start(b0S[:], b0s[:])

            r0S = wp.tile([128, KC, SH], BF16)
            nc.sync.dma_start(r0S[:], r0s[:].rearrange("(c p) n -> p c n", p=128))
            k1S = wp.tile([128, KC, SH], BF16)
            nc.sync.dma_start(k1S[:], k1s[:].rearrange("(c p) n -> p c n", p=128))
            r1S = wp.tile([128, KC, SH], BF16)
            nc.sync.dma_start(r1S[:], r1s[:].rearrange("(c p) n -> p c n", p=128))
            k0S = wp.tile([128, KC0, SH], BF16)
            nc.sync.dma_start(k0S[:], k0s[:].rearrange("(c p) n -> p c n", p=128))
            woutS = wp.tile([128, KC, OUTD], BF16)
            nc.sync.dma_start(woutS[:], wout[:].rearrange("(c p) n -> p c n", p=128))

            # gathered hT ring tiles (persistent; halves/slots written per step)
            hr1 = [pp.tile([128, KC, 128], BF16, name=f"hr1_{i}") for i in range(3)]
            hr2 = [pp.tile([128, KC, B], BF16, name=f"hr2_{i}") for i in range(3)]
            # premult-row pair-gather ring (rows 0:64 = even step,
            # 64:128 = odd step of the pair)
            g4 = [pp.tile([128, SH], BF16, name=f"g4_{i}") for i in range(RB)]

            # --- phase 1: premult = emb @ k0_shard + b0_shard (+ row-0 mask
            # pattern: token 0 is the masked token) ---
            onesS = pp.tile([1, 128], BF16)
            nc.vector.memset(onesS[:], 1.0)
            with (
                tc.tile_pool(name="pm_sb", bufs=4) as pmsb,
                tc.tile_pool(name="pm_ps", bufs=2, space="PSUM") as pmps,
            ):
                for v in range(VT):
                    et = pmsb.tile([128, KC0, 128], BF16, tag="embtile")
                    nc.sync.dma_start(
                        et[:], embT[:, v * 128 : (v + 1) * 128].rearrange(
                            "(c p) n -> p c n", p=128
                        )
                    )
                    ps = pmps.tile([128, SH], F32)
                    nc.tensor.matmul(
                        ps[:], onesS[:1, :], b0S[:1, :], start=True, stop=False
                    )
                    for c in range(KC0):
                        nc.tensor.matmul(
                            ps[:], et[:, c, :], k0S[:, c, :],
                            start=False, stop=(c == KC0 - 1),
                        )
                    pv = pmsb.tile([128, SH], BF16, tag="pmtile")
                    nc.vector.tensor_copy(pv[:], ps[:])
                    if v == 0:
                        # masked-token row: fold the i/f forcing pattern in
                        nc.vector.tensor_tensor(
                            pv[0:1, :], ps[0:1, :], patfS[0:1, :], OP.add
                        )
                    nc.sync.dma_start(premult[v * 128 : (v + 1) * 128, :], pv[:])

            # --- phase 2: recurrences (layer 2 lags by LAG steps) ---
            with (
                tc.tile_pool(name="state", bufs=2) as st,
                tc.tile_pool(name="gates", bufs=3) as gt,
                tc.tile_pool(name="zps", bufs=2, space="PSUM") as zps,
                tc.tile_pool(name="xzps", bufs=3, space="PSUM") as xzps,
                tc.tile_pool(name="trps", bufs=1, space="PSUM") as trps,
                tc.tile_pool(name="wps", bufs=1, space="PSUM") as wps,
            ):
                c1 = st.tile([B, HSH], F32, tag="c1")
                h1 = st.tile([B, HSH], BF16, tag="h1")
                c2 = st.tile([B, HSH], F32, tag="c2")
                h2 = st.tile([B, HSH], BF16, tag="h2")
                for tl in (c1, h1, c2, h2):
                    nc.vector.memset(tl[:], 0.0)

                xz_tiles = {}  # pair index (s//2) -> psum tile

                def cell(zh, li, t_idx, c_old, h_old, trs_dst):
                    """Gates+cell+mask for one layer step.

                    zh: [64, 512] psum slice holding z (+/-30 i/f pattern
                    already folded in for masked steps). Returns
                    (c_new, h_new, trs) where trs is h_new^T in SBUF.
                    """
                    mcol = mS[:, t_idx : t_idx + 1]
                    sg = gt.tile([B, 384], F32, tag=f"sg{li}")
                    nc.scalar.activation(sg[:], zh[:, 0:384], AF.Sigmoid)
                    gg = gt.tile([B, HSH], BF16, tag=f"gg{li}")
                    nc.scalar.activation(gg[:], zh[:, 384:512], AF.Tanh)
                    u = gt.tile([B, HSH], BF16, tag=f"u{li}")
                    nc.vector.tensor_tensor(u[:], sg[:, 0:128], gg[:], OP.mult)
                    v = gt.tile([B, HSH], F32, tag=f"v{li}")
                    # on the (otherwise idle) Pool engine: runs parallel to u
                    nc.gpsimd.tensor_tensor(v[:], sg[:, 128:256], c_old[:], OP.mult)
                    c_new = st.tile([B, HSH], F32, tag=f"c{li}")
                    nc.vector.tensor_tensor(c_new[:], u[:], v[:], OP.add)
                    th = gt.tile([B, HSH], BF16, tag=f"th{li}")
                    nc.scalar.activation(th[:], c_new[:], AF.Tanh)
                    hn = gt.tile([B, HSH], BF16, tag=f"hn{li}")
                    nc.vector.tensor_tensor(hn[:], sg[:, 256:384], th[:], OP.mult)
                    dh = gt.tile([B, HSH], BF16, tag=f"dh{li}")
                    nc.vector.tensor_tensor(dh[:], hn[:], h_old[:], OP.subtract)
                    h_new = st.tile([B, HSH], BF16, tag=f"h{li}")
                    nc.vector.scalar_tensor_tensor(
                        h_new[:], dh[:], mcol, h_old[:], OP.mult, OP.add
                    )
                    trp = trps.tile([HSH, B], BF16, tag=f"tr{li}")
                    nc.tensor.transpose(trp[:], h_new[:], eye64S[:])
                    nc.vector.tensor_copy(trs_dst, trp[:])
                    return c_new, h_new

                def hT1_slice(step, c):
                    j = (step // 2) % 3
                    off = 64 * (step % 2)
                    return hr1[j][:, c, off : off + 64]

                for t in range(t_steps + LAG):
                    slot = t % RB
                    p = t % 2
                    has_z1 = t < t_steps
                    has_l2 = t >= LAG
                    s = t - LAG
                    sp = s % 2

                    # ---- gather premult rows two step-pairs ahead so the
                    # gather is immune to GpSimd issue-queue jitter (ring of
                    # 4 holds pairs P..P+3) ----
                    if t == 0:
                        gps = [0, 1]
                    elif t % 2 == 1:
                        gps = [(t + 3) // 2]
                    else:
                        gps = []
                    for gpair in gps:
                        if gpair < t_steps // 2:
                            nc.gpsimd.indirect_dma_start(
                                out=g4[gpair % RB][:],
                                out_offset=None,
                                in_=premult[:],
                                in_offset=bass.IndirectOffsetOnAxis(
                                    ap=tokPS[:, gpair : gpair + 1], axis=0
                                ),
                            )
                    if has_z1:
                        gtile = g4[(t // 2) % RB]
                        zt = zps.tile([128, SH], F32, tag="z1")
                        z1 = zt[64 * p : 64 * p + 64, :]
                    if has_l2:
                        xt2 = xz_tiles[s // 2]
                        z2 = xt2[64 * sp : 64 * sp + 64, :]

                    # ---- z1 (col tile p) and r1 (col tile 1-p) matmuls,
                    # emitted interleaved so the two 128x64 array tiles
                    # stream concurrently (MMs issue in program order) ----
                    if has_z1:
                        eyeh = eye128S if p == 0 else eyebotS
                        nc.tensor.matmul(
                            z1, eyeh[:], gtile[:], start=True, stop=(t == 0)
                        )
                    for c in range(KC):
                        if has_z1 and t > 0:
                            nc.tensor.matmul(
                                z1, hT1_slice(t - 1, c), r0S[:, c, :],
                                start=False, stop=(c == KC - 1),
                            )
                        if has_l2 and s > 0:
                            nc.tensor.matmul(
                                z2, hr2[(t - 1) % 3][:, c, :], r1S[:, c, :],
                                start=False, stop=(c == KC - 1),
                                skip_group_check=True,
                            )

                    # shared staging tile: h1T | h2T side by side, ONE DMA out
                    trsP = gt.tile([HSH, 2 * B], BF16, tag="trsP")
                    if has_z1:
                        # ---- layer-1 cell ----
                        c1, h1 = cell(z1, 1, t, c1, h1, trsP[:, 0:B])

                    if has_l2:
                        # ---- layer-2 cell for step s ----
                        c2, h2 = cell(z2, 2, s, c2, h2, trsP[:, B : 2 * B])
                        if sp == 1:
                            xz_tiles.pop(s // 2, None)

                    nc.sync.dma_start(agin[slot][:], trsP[:])

                    # ---- ONE combined h1|h2 AllGather per iteration ----
                    nc.gpsimd.collective_compute(
                        "AllGather",
                        OP.bypass,
                        replica_groups=[list(range(NC))],
                        ins=[agin[slot][:].opt()],
                        outs=[agout[slot][:].opt()],
                    )
                    if has_z1:
                        nc.sync.dma_start(
                            hr1[(t // 2) % 3][:, :, 64 * p : 64 * p + 64],
                            agout[slot][:, 0:B].rearrange(
                                "(c p) n -> p c n", p=128
                            ),
                        )
                    if has_l2:
                        # scalar-queue HWDGE: issues in parallel with the
                        # hr1 split DMA on the sync queue
                        nc.scalar.dma_start(
                            hr2[t % 3][:],
                            agout[slot][:, B : 2 * B].rearrange(
                                "(c p) n -> p c n", p=128
                            ),
                        )

                    if t >= 2 and t % 2 == 0 and t <= t_steps:
                        # ---- batched layer-2 input projection for steps
                        # (t-2, t-1) at full array width ----
                        pair = (t - 2) // 2
                        xt = xzps.tile([128, SH], F32, tag="xz2")
                        xz_tiles[pair] = xt
                        # K=2 stationary [30*(1-mask) pair row; ones row]
                        # with rhs [i/f pattern; b1]: adds mask forcing AND
                        # the layer-2 bias in one matmul
                        mrow = m30Cs[pair // NP4][0:2, pair % NP4, :]
                        nc.tensor.matmul(
                            xt[:], mrow, patbS[0:2, :],
                            start=True, stop=False, skip_group_check=True,
                        )
                        for c in range(KC):
                            nc.tensor.matmul(
                                xt[:], hr1[pair % 3][:, c, :], k1S[:, c, :],
                                start=False, stop=(c == KC - 1),
                                skip_group_check=True,
                            )

                # ---- output: logits = h2_final @ Wout + bout ----
                tf = (t_steps + LAG - 1) % 3
                ops = wps.tile([B, OUTD], F32, tag="ops")
                nc.tensor.matmul(
                    ops[:], onesS[:1, :B], boutS[:1, :], start=True, stop=False
                )
                for c in range(KC):
                    nc.tensor.matmul(
                        ops[:], hr2[tf][:, c, :], woutS[:, c, :],
                        start=False, stop=(c == KC - 1),
                    )
                logits = gt.tile([B, OUTD], F32, tag="logits")
                nc.vector.tensor_copy(logits[:], ops[:])
                nc.sync.dma_start(out[:], logits[:])

    nc.finalize()
    return nc


_CACHE = {}


def _get_nc(t_steps):
    if t_steps not in _CACHE:
        _CACHE[t_steps] = build(t_steps)
    return _CACHE[t_steps]


def _prep_in_maps(inputs, emb, k0, r0, b0, k1, r1, b1, Wout, bout):
    bf = ml_dtypes.bfloat16
    embT = np.ascontiguousarray(emb.T).astype(bf)
    tok = np.ascontiguousarray(inputs.astype(np.int32))
    t_steps = int(os.environ.get("LSTM_T", T))

    eye64 = np.eye(B, dtype=bf)
    eye128 = np.zeros((128, B), dtype=bf)
    eye128[:B] = np.eye(B)

    m = (tok != 0).astype(np.float32)  # [B, T]
    m30 = 30.0 * (1.0 - m)  # [B, T]
    # step-pair mask rows + ones rows: partition 0 = [m30[:,2P] | m30[:,2P+1]],
    # partition 1 = ones (carries b1 through the K=2 pattern matmul)
    m30C = np.stack(
        [m30.T.reshape(T // 2, 2 * B), np.ones((T // 2, 2 * B), np.float32)]
    ).astype(bf)
    # pair-major token indices: rows 0:64 = step 2P, rows 64:128 = step 2P+1
    tokP = np.ascontiguousarray(
        np.concatenate([tok[:, 0::2], tok[:, 1::2]], axis=0)
    ).astype(np.int32)
    eyebot = np.zeros((128, B), dtype=bf)
    eyebot[B:] = np.eye(B)

    pat1 = np.zeros((1, SH), dtype=bf)
    pat1[0, 0:128] = -1.0
    pat1[0, 128:256] = 1.0
    patf = np.zeros((1, SH), dtype=np.float32)
    patf[0, 0:128] = -30.0
    patf[0, 128:256] = 30.0

    in_maps = []
    for c in range(NC):
        hc = slice(c * HSH, (c + 1) * HSH)
        # per-core gate-column permutation: [i | f | o | g] blocks
        cols = np.concatenate(
            [
                np.arange(0 * HID, 1 * HID)[hc],  # i
                np.arange(1 * HID, 2 * HID)[hc],  # f
                np.arange(3 * HID, 4 * HID)[hc],  # o
                np.arange(2 * HID, 3 * HID)[hc],  # g
            ]
        )
        b1c = np.ascontiguousarray(b1[cols]).astype(np.float32)
        patb = np.stack([pat1[0].astype(np.float32), b1c]).astype(bf)
        in_maps.append(
            {
                "tok": tok,
                "embT": embT,
                "k0s": np.ascontiguousarray(k0[:, cols]).astype(bf),
                "r0s": np.ascontiguousarray(r0[:, cols]).astype(bf),
                "k1s": np.ascontiguousarray(k1[:, cols]).astype(bf),
                "r1s": np.ascontiguousarray(r1[:, cols]).astype(bf),
                "b0s": np.ascontiguousarray(b0[cols])[None, :].astype(bf),
                "patb": patb,
                "wout": np.ascontiguousarray(Wout).astype(bf),
                "bout": np.ascontiguousarray(bout)[None, :].astype(bf),
                "eye128": eye128,
                "eyebot": eyebot,
                "tokP": tokP,
                "eye64": eye64,
                "m30C": m30C,
                "mflt": m,

                "pat1": pat1,
                "patf": patf,
            }
        )
    return in_maps


def kernel(inputs, emb, k0, r0, b0, k1, r1, b1, Wout, bout, _trace=False):
    t_steps = int(os.environ.get("LSTM_T", T))
    nc = _get_nc(t_steps)
    in_maps = _prep_in_maps(
        np.asarray(inputs), np.asarray(emb), np.asarray(k0), np.asarray(r0),
        np.asarray(b0), np.asarray(k1), np.asarray(r1), np.asarray(b1),
        np.asarray(Wout), np.asarray(bout),
    )
    res = run_bass_kernel_spmd(
        nc, in_maps, core_ids=list(range(NC)), trace=_trace
    )
    kernel.last_result = res
    return res.results[0]["out"].astype(np.float32)



# revision 14
# speedup vs baseline: 1.0251x; 1.0151x over previous
"""Trainium2 Bass kernel for nn_ClassifierLSTM (2-layer masked LSTM classifier).

Tensor-parallel over the gate dimension across 8 NeuronCores: each core owns
128 hidden units (512 gate columns arranged [i|f|o|g]) of both LSTM layers.

Design (vs. the serialized interleave-by-1 baseline):
 - 2x PE column tiling: z1 accumulates in psum partitions [64p, 64p+64)
   (p = step parity) while layer-2's recurrent (r1) matmuls accumulate in
   the opposite half of the xz2 batch tile; the two matmul streams are
   emitted interleaved so the two 128x64 array tiles stream concurrently
   (matmuls issue strictly in program order).
 - layer 2 lags layer 1 by 3 steps; its input projection xz2 = h1 @ k1 + b1
   (+ mask pattern) is computed in M=128 batches of TWO timesteps at full
   array width into a psum ring the r1 recurrence accumulates into. The
   batch matmuls are emitted at the iteration tail so they fill the
   collective-wait window instead of delaying the h-exchange fires.
 - ONE combined AllGather per step: h1T(t) and h2T(t-LAG) are staged side
   by side in a single SBUF tile [HSH, 2B] by the two cells' transpose
   copies, written to DRAM with a single DMA (one writer -> the collective
   reliably waits on it; two half-writes raced under Tile's per-tensor
   last-writer DRAM tracking), gathered once, and split back into the hr1
   / hr2 rings by two column-sliced DMAs. Halves the per-step ncfw
   collective floor (~10us -> ~7.5us busy, Comms occupancy 62% -> 30%).
   Premult rows for two steps are fetched by one indirect DMA via
   a pair-major token index, and the per-step inject selects the pair half
   with a top/bottom identity stationary (stays column-tile paired).
 - Keras mask_zero folded into the gate inputs: premult row 0 (the masked
   token's row) carries a [-30|+30|0|0] i/f pattern; xz2 batches get the
   pattern AND b1 via one K=2 matmul ([mask row; ones row] stationary).
   Cells then need only 3 activation instructions each (sigmoid over
   [i|f|o], tanh(g), tanh(c)).
 - gate tensors bf16 where safe (sigmoid outputs stay f32 to protect the
   512-step cell-state recursion) for 2x DVE throughput.

Caution: matmul stationary reads at free-dim offsets beyond ~32KB into a
partition silently return garbage — keep per-step lookup tables split into
small tiles (m30C is 4x 16KB).
"""
import os
import sys

sys.path.insert(0, "/opt/trn_rl_repo")

import numpy as np
import ml_dtypes

import concourse.bass as bass
import concourse.mybir as mybir
import concourse.tile as tile
from concourse import bacc
from concourse.bass_utils import run_bass_kernel_spmd

F32 = mybir.dt.float32
BF16 = mybir.dt.bfloat16
I32 = mybir.dt.int32
AF = mybir.ActivationFunctionType
OP = mybir.AluOpType

VOCAB, EMB, HID, OUTD = 32000, 512, 1024, 3
B, T = 64, 512
NC = 8
SH = 512  # gate columns per core (= 4 * 128 hidden units)
HSH = 128  # hidden units per core
KC = HID // 128  # 8 K-chunks for recurrent / layer-2 matmuls
KC0 = EMB // 128  # 4 K-chunks for the premult matmul
VT = VOCAB // 128  # 250 vocab tiles
RB = 4  # bounce-buffer ring depth
LAG = 3  # layer-2 step lag behind layer 1


def build(t_steps=T):
    assert t_steps % 2 == 0, "k1 batching assumes an even number of steps"
    nc = bacc.Bacc("TRN2", target_bir_lowering=False, debug=False, num_devices=NC)

    # ---- I/O ----
    tok = nc.dram_tensor("tok", [B, T], I32, kind="ExternalInput")
    embT = nc.dram_tensor("embT", [EMB, VOCAB], BF16, kind="ExternalInput")
    k0s = nc.dram_tensor("k0s", [EMB, SH], BF16, kind="ExternalInput")
    r0s = nc.dram_tensor("r0s", [HID, SH], BF16, kind="ExternalInput")
    k1s = nc.dram_tensor("k1s", [HID, SH], BF16, kind="ExternalInput")
    r1s = nc.dram_tensor("r1s", [HID, SH], BF16, kind="ExternalInput")
    b0s = nc.dram_tensor("b0s", [1, SH], BF16, kind="ExternalInput")
    patb = nc.dram_tensor("patb", [2, SH], BF16, kind="ExternalInput")
    wout = nc.dram_tensor("wout", [HID, OUTD], BF16, kind="ExternalInput")
    bout = nc.dram_tensor("bout", [1, OUTD], BF16, kind="ExternalInput")
    eye128 = nc.dram_tensor("eye128", [128, B], BF16, kind="ExternalInput")
    eyebot = nc.dram_tensor("eyebot", [128, B], BF16, kind="ExternalInput")
    tokP = nc.dram_tensor("tokP", [128, T // 2], I32, kind="ExternalInput")
    eye64 = nc.dram_tensor("eye64", [B, B], BF16, kind="ExternalInput")
    m30C = nc.dram_tensor("m30C", [2, T // 2, 2 * B], BF16, kind="ExternalInput")
    mflt = nc.dram_tensor("mflt", [B, T], F32, kind="ExternalInput")
    pat1 = nc.dram_tensor("pat1", [1, SH], BF16, kind="ExternalInput")
    patf = nc.dram_tensor("patf", [1, SH], F32, kind="ExternalInput")
    out = nc.dram_tensor("out", [B, OUTD], F32, kind="ExternalOutput")

    # ---- internal DRAM ----
    premult = nc.dram_tensor("premult", [VOCAB, SH], BF16)
    # combined h1|h2 exchange: one AllGather per step of [HSH, 2B]
    # (h1T in cols 0:B, h2T in cols B:2B)
    agin = [nc.dram_tensor(f"agin_{k}", [HSH, 2 * B], BF16) for k in range(RB)]
    agout = [
        nc.dram_tensor(f"agout_{k}", [NC * HSH, 2 * B], BF16, addr_space="Shared")
        for k in range(RB)
    ]


    with tile.TileContext(nc) as tc:
        with (
            tc.tile_pool(name="persist", bufs=1) as pp,
            tc.tile_pool(name="wpool", bufs=1) as wp,
        ):
            # --- resident tiles ---
            tokS = pp.tile([B, T], I32)
            nc.sync.dma_start(tokS[:], tok[:])
            mS = pp.tile([B, T], F32)
            nc.sync.dma_start(mS[:], mflt[:])
            # mask+ones stationary pairs, in 64-pair tiles so
            # stationary-AP free offsets stay small (<16KB)
            NP4 = T // 8
            m30Cs = []
            for k in range(4):
                mt = pp.tile([2, NP4, 2 * B], BF16, name=f"m30C_{k}")
                nc.sync.dma_start(mt[:], m30C[:, k * NP4 : (k + 1) * NP4, :])
                m30Cs.append(mt)
            tokPS = pp.tile([128, T // 2], I32)
            nc.sync.dma_start(tokPS[:], tokP[:])

            eye128S = pp.tile([128, B], BF16)
            nc.sync.dma_start(eye128S[:], eye128[:])
            eyebotS = pp.tile([128, B], BF16)
            nc.sync.dma_start(eyebotS[:], eyebot[:])
            eye64S = pp.tile([B, B], BF16)
            nc.sync.dma_start(eye64S[:], eye64[:])
            patbS = pp.tile([2, SH], BF16)
            nc.sync.dma_start(patbS[:], patb[:])
            pat1S = pp.tile([1, SH], BF16)
            nc.sync.dma_start(pat1S[:], pat1[:])
            patfS = pp.tile([1, SH], F32)
            nc.sync.dma_start(patfS[:], patf[:])
            boutS = pp.tile([1, OUTD], BF16)
            nc.sync.dma_start(boutS[:], bout[:])
            b0S = pp.tile([1, SH], BF16)
            nc.sync.dma_start(b0S[:], b0s[:])

            r0S = wp.tile([128, KC, SH], BF16)
            nc.sync.dma_start(r0S[:], r0s[:].rearrange("(c p) n -> p c n", p=128))
            k1S = wp.tile([128, KC, SH], BF16)
            nc.sync.dma_start(k1S[:], k1s[:].rearrange("(c p) n -> p c n", p=128))
            r1S = wp.tile([128, KC, SH], BF16)
            nc.sync.dma_start(r1S[:], r1s[:].rearrange("(c p) n -> p c n", p=128))
            k0S = wp.tile([128, KC0, SH], BF16)
            nc.sync.dma_start(k0S[:], k0s[:].rearrange("(c p) n -> p c n", p=128))
            woutS = wp.tile([128, KC, OUTD], BF16)
            nc.sync.dma_start(woutS[:], wout[:].rearrange("(c p) n -> p c n", p=128))

            # gathered hT ring tiles (persistent; halves/slots written per step)
            hr1 = [pp.tile([128, KC, 128], BF16, name=f"hr1_{i}") for i in range(3)]
            hr2 = [pp.tile([128, KC, B], BF16, name=f"hr2_{i}") for i in range(3)]
            # premult-row pair-gather ring (rows 0:64 = even step,
            # 64:128 = odd step of the pair)
            g4 = [pp.tile([128, SH], BF16, name=f"g4_{i}") for i in range(RB)]

            # --- phase 1: premult = emb @ k0_shard + b0_shard (+ row-0 mask
            # pattern: token 0 is the masked token) ---
            onesS = pp.tile([1, 128], BF16)
            nc.vector.memset(onesS[:], 1.0)
            with (
                tc.tile_pool(name="pm_sb", bufs=4) as pmsb,
                tc.tile_pool(name="pm_ps", bufs=2, space="PSUM") as pmps,
            ):
                for v in range(VT):
                    et = pmsb.tile([128, KC0, 128], BF16, tag="embtile")
                    nc.sync.dma_start(
                        et[:], embT[:, v * 128 : (v + 1) * 128].rearrange(
                            "(c p) n -> p c n", p=128
                        )
                    )
                    ps = pmps.tile([128, SH], F32)
                    nc.tensor.matmul(
                        ps[:], onesS[:1, :], b0S[:1, :], start=True, stop=False
                    )
                    for c in range(KC0):
                        nc.tensor.matmul(
                            ps[:], et[:, c, :], k0S[:, c, :],
                            start=False, stop=(c == KC0 - 1),
                        )
                    pv = pmsb.tile([128, SH], BF16, tag="pmtile")
                    nc.vector.tensor_copy(pv[:], ps[:])
                    if v == 0:
                        # masked-token row: fold the i/f forcing pattern in
                        nc.vector.tensor_tensor(
                            pv[0:1, :], ps[0:1, :], patfS[0:1, :], OP.add
                        )
                    nc.sync.dma_start(premult[v * 128 : (v + 1) * 128, :], pv[:])

            # --- phase 2: recurrences (layer 2 lags by LAG steps) ---
            with (
                tc.tile_pool(name="state", bufs=2) as st,
                tc.tile_pool(name="gates", bufs=3) as gt,
                tc.tile_pool(name="zps", bufs=2, space="PSUM") as zps,
                tc.tile_pool(name="xzps", bufs=3, space="PSUM") as xzps,
                tc.tile_pool(name="trps", bufs=1, space="PSUM") as trps,
                tc.tile_pool(name="wps", bufs=1, space="PSUM") as wps,
            ):
                c1 = st.tile([B, HSH], F32, tag="c1")
                h1 = st.tile([B, HSH], BF16, tag="h1")
                c2 = st.tile([B, HSH], F32, tag="c2")
                h2 = st.tile([B, HSH], BF16, tag="h2")
                for tl in (c1, h1, c2, h2):
                    nc.vector.memset(tl[:], 0.0)

                xz_tiles = {}  # pair index (s//2) -> psum tile

                def cells_pair(specs):
                    """Gates+cell+mask for 1-2 layer steps, phase-interleaved.

                    Each spec: (zh [64,512] psum slice with +/-30 i/f pattern
                    folded in for masked steps, li, t_idx, c_old, h_old,
                    trs_dst). Ops of the two cells are emitted phase by phase
                    so their chains software-pipeline across the ACT/DVE
                    queues (Tile enforces the data deps either way; emission
                    order is the per-engine schedule). Returns
                    [(c_new, h_new), ...].
                    """
                    sgs, ggs, us, vs, cns, ths, hns = [], [], [], [], [], [], []
                    for (zh, li, t_idx, c_old, h_old, trs_dst) in specs:
                        sg = gt.tile([B, 384], F32, tag=f"sg{li}")
                        nc.scalar.activation(sg[:], zh[:, 0:384], AF.Sigmoid)
                        gg = gt.tile([B, HSH], BF16, tag=f"gg{li}")
                        nc.scalar.activation(gg[:], zh[:, 384:512], AF.Tanh)
                        sgs.append(sg)
                        ggs.append(gg)
                    for i, (zh, li, t_idx, c_old, h_old, trs_dst) in enumerate(
                        specs
                    ):
                        u = gt.tile([B, HSH], BF16, tag=f"u{li}")
                        nc.vector.tensor_tensor(
                            u[:], sgs[i][:, 0:128], ggs[i][:], OP.mult
                        )
                        v = gt.tile([B, HSH], F32, tag=f"v{li}")
                        # on the (otherwise idle) Pool engine: parallel to u
                        nc.gpsimd.tensor_tensor(
                            v[:], sgs[i][:, 128:256], c_old[:], OP.mult
                        )
                        us.append(u)
                        vs.append(v)
                    for i, (zh, li, t_idx, c_old, h_old, trs_dst) in enumerate(
                        specs
                    ):
                        c_new = st.tile([B, HSH], F32, tag=f"c{li}")
                        nc.vector.tensor_tensor(
                            c_new[:], us[i][:], vs[i][:], OP.add
                        )
                        cns.append(c_new)
                    for i, (zh, li, t_idx, c_old, h_old, trs_dst) in enumerate(
                        specs
                    ):
                        th = gt.tile([B, HSH], BF16, tag=f"th{li}")
                        nc.scalar.activation(th[:], cns[i][:], AF.Tanh)
                        ths.append(th)
                    outs = []
                    for i, (zh, li, t_idx, c_old, h_old, trs_dst) in enumerate(
                        specs
                    ):
                        mcol = mS[:, t_idx : t_idx + 1]
                        hn = gt.tile([B, HSH], BF16, tag=f"hn{li}")
                        nc.vector.tensor_tensor(
                            hn[:], sgs[i][:, 256:384], ths[i][:], OP.mult
                        )
                        dh = gt.tile([B, HSH], BF16, tag=f"dh{li}")
                        nc.vector.tensor_tensor(
                            dh[:], hn[:], h_old[:], OP.subtract
                        )
                        h_new = st.tile([B, HSH], BF16, tag=f"h{li}")
                        nc.vector.scalar_tensor_tensor(
                            h_new[:], dh[:], mcol, h_old[:], OP.mult, OP.add
                        )
                        trp = trps.tile([HSH, B], BF16, tag=f"tr{li}")
                        nc.tensor.transpose(trp[:], h_new[:], eye64S[:])
                        nc.vector.tensor_copy(trs_dst, trp[:])
                        outs.append((cns[i], h_new))
                    return outs

                def hT1_slice(step, c):
                    j = (step // 2) % 3
                    off = 64 * (step % 2)
                    return hr1[j][:, c, off : off + 64]

                for t in range(t_steps + LAG):
                    slot = t % RB
                    p = t % 2
                    has_z1 = t < t_steps
                    has_l2 = t >= LAG
                    s = t - LAG
                    sp = s % 2

                    # ---- gather premult rows two step-pairs ahead so the
                    # gather is immune to GpSimd issue-queue jitter (ring of
                    # 4 holds pairs P..P+3) ----
                    if t == 0:
                        gps = [0, 1]
                    elif t % 2 == 1:
                        gps = [(t + 3) // 2]
                    else:
                        gps = []
                    for gpair in gps:
                        if gpair < t_steps // 2:
                            nc.gpsimd.indirect_dma_start(
                                out=g4[gpair % RB][:],
                                out_offset=None,
                                in_=premult[:],
                                in_offset=bass.IndirectOffsetOnAxis(
                                    ap=tokPS[:, gpair : gpair + 1], axis=0
                                ),
                            )
                    if has_z1:
                        gtile = g4[(t // 2) % RB]
                        zt = zps.tile([128, SH], F32, tag="z1")
                        z1 = zt[64 * p : 64 * p + 64, :]
                    if has_l2:
                        xt2 = xz_tiles[s // 2]
                        z2 = xt2[64 * sp : 64 * sp + 64, :]

                    # ---- z1 (col tile p) and r1 (col tile 1-p) matmuls,
                    # emitted interleaved so the two 128x64 array tiles
                    # stream concurrently (MMs issue in program order) ----
                    if has_z1:
                        eyeh = eye128S if p == 0 else eyebotS
                        nc.tensor.matmul(
                            z1, eyeh[:], gtile[:], start=True, stop=(t == 0)
                        )
                    for c in range(KC):
                        if has_z1 and t > 0:
                            nc.tensor.matmul(
                                z1, hT1_slice(t - 1, c), r0S[:, c, :],
                                start=False, stop=(c == KC - 1),
                            )
                        if has_l2 and s > 0:
                            nc.tensor.matmul(
                                z2, hr2[(t - 1) % 3][:, c, :], r1S[:, c, :],
                                start=False, stop=(c == KC - 1),
                                skip_group_check=True,
                            )

                    # shared staging tile: h1T | h2T side by side, ONE DMA out
                    trsP = gt.tile([HSH, 2 * B], BF16, tag="trsP")
                    specs = []
                    if has_z1:
                        specs.append((z1, 1, t, c1, h1, trsP[:, 0:B]))
                    if has_l2:
                        specs.append((z2, 2, s, c2, h2, trsP[:, B : 2 * B]))
                    res = cells_pair(specs)
                    if has_z1:
                        c1, h1 = res[0]
                    if has_l2:
                        c2, h2 = res[-1]
                        if sp == 1:
                            xz_tiles.pop(s // 2, None)

                    nc.sync.dma_start(agin[slot][:], trsP[:])

                    # ---- ONE combined h1|h2 AllGather per iteration ----
                    nc.gpsimd.collective_compute(
                        "AllGather",
                        OP.bypass,
                        replica_groups=[list(range(NC))],
                        ins=[agin[slot][:].opt()],
                        outs=[agout[slot][:].opt()],
                    )
                    if has_z1:
                        nc.sync.dma_start(
                            hr1[(t // 2) % 3][:, :, 64 * p : 64 * p + 64],
                            agout[slot][:, 0:B].rearrange(
                                "(c p) n -> p c n", p=128
                            ),
                        )
                    if has_l2:
                        # scalar-queue HWDGE: issues in parallel with the
                        # hr1 split DMA on the sync queue
                        nc.scalar.dma_start(
                            hr2[t % 3][:],
                            agout[slot][:, B : 2 * B].rearrange(
                                "(c p) n -> p c n", p=128
                            ),
                        )

                    if t >= 2 and t % 2 == 0 and t <= t_steps:
                        # ---- batched layer-2 input projection for steps
                        # (t-2, t-1) at full array width ----
                        pair = (t - 2) // 2
                        xt = xzps.tile([128, SH], F32, tag="xz2")
                        xz_tiles[pair] = xt
                        # K=2 stationary [30*(1-mask) pair row; ones row]
                        # with rhs [i/f pattern; b1]: adds mask forcing AND
                        # the layer-2 bias in one matmul
                        mrow = m30Cs[pair // NP4][0:2, pair % NP4, :]
                        nc.tensor.matmul(
                            xt[:], mrow, patbS[0:2, :],
                            start=True, stop=False, skip_group_check=True,
                        )
                        for c in range(KC):
                            nc.tensor.matmul(
                                xt[:], hr1[pair % 3][:, c, :], k1S[:, c, :],
                                start=False, stop=(c == KC - 1),
                                skip_group_check=True,
                            )

                # ---- output: logits = h2_final @ Wout + bout ----
                tf = (t_steps + LAG - 1) % 3
                ops = wps.tile([B, OUTD], F32, tag="ops")
                nc.tensor.matmul(
                    ops[:], onesS[:1, :B], boutS[:1, :], start=True, stop=False
                )
                for c in range(KC):
                    nc.tensor.matmul(
                        ops[:], hr2[tf][:, c, :], woutS[:, c, :],
                        start=False, stop=(c == KC - 1),
                    )
                logits = gt.tile([B, OUTD], F32, tag="logits")
                nc.vector.tensor_copy(logits[:], ops[:])
                nc.sync.dma_start(out[:], logits[:])

    nc.finalize()
    return nc


_CACHE = {}


def _get_nc(t_steps):
    if t_steps not in _CACHE:
        _CACHE[t_steps] = build(t_steps)
    return _CACHE[t_steps]


def _prep_in_maps(inputs, emb, k0, r0, b0, k1, r1, b1, Wout, bout):
    bf = ml_dtypes.bfloat16
    embT = np.ascontiguousarray(emb.T).astype(bf)
    tok = np.ascontiguousarray(inputs.astype(np.int32))
    t_steps = int(os.environ.get("LSTM_T", T))

    eye64 = np.eye(B, dtype=bf)
    eye128 = np.zeros((128, B), dtype=bf)
    eye128[:B] = np.eye(B)

    m = (tok != 0).astype(np.float32)  # [B, T]
    m30 = 30.0 * (1.0 - m)  # [B, T]
    # step-pair mask rows + ones rows: partition 0 = [m30[:,2P] | m30[:,2P+1]],
    # partition 1 = ones (carries b1 through the K=2 pattern matmul)
    m30C = np.stack(
        [m30.T.reshape(T // 2, 2 * B), np.ones((T // 2, 2 * B), np.float32)]
    ).astype(bf)
    # pair-major token indices: rows 0:64 = step 2P, rows 64:128 = step 2P+1
    tokP = np.ascontiguousarray(
        np.concatenate([tok[:, 0::2], tok[:, 1::2]], axis=0)
    ).astype(np.int32)
    eyebot = np.zeros((128, B), dtype=bf)
    eyebot[B:] = np.eye(B)

    pat1 = np.zeros((1, SH), dtype=bf)
    pat1[0, 0:128] = -1.0
    pat1[0, 128:256] = 1.0
    patf = np.zeros((1, SH), dtype=np.float32)
    patf[0, 0:128] = -30.0
    patf[0, 128:256] = 30.0

    in_maps = []
    for c in range(NC):
        hc = slice(c * HSH, (c + 1) * HSH)
        # per-core gate-column permutation: [i | f | o | g] blocks
        cols = np.concatenate(
            [
                np.arange(0 * HID, 1 * HID)[hc],  # i
                np.arange(1 * HID, 2 * HID)[hc],  # f
                np.arange(3 * HID, 4 * HID)[hc],  # o
                np.arange(2 * HID, 3 * HID)[hc],  # g
            ]
        )
        b1c = np.ascontiguousarray(b1[cols]).astype(np.float32)
        patb = np.stack([pat1[0].astype(np.float32), b1c]).astype(bf)
        in_maps.append(
            {
                "tok": tok,
                "embT": embT,
                "k0s": np.ascontiguousarray(k0[:, cols]).astype(bf),
                "r0s": np.ascontiguousarray(r0[:, cols]).astype(bf),
                "k1s": np.ascontiguousarray(k1[:, cols]).astype(bf),
                "r1s": np.ascontiguousarray(r1[:, cols]).astype(bf),
                "b0s": np.ascontiguousarray(b0[cols])[None, :].astype(bf),
                "patb": patb,
                "wout": np.ascontiguousarray(Wout).astype(bf),
                "bout": np.ascontiguousarray(bout)[None, :].astype(bf),
                "eye128": eye128,
                "eyebot": eyebot,
                "tokP": tokP,
                "eye64": eye64,
                "m30C": m30C,
                "mflt": m,

                "pat1": pat1,
                "patf": patf,
            }
        )
    return in_maps


def kernel(inputs, emb, k0, r0, b0, k1, r1, b1, Wout, bout, _trace=False):
    t_steps = int(os.environ.get("LSTM_T", T))
    nc = _get_nc(t_steps)
    in_maps = _prep_in_maps(
        np.asarray(inputs), np.asarray(emb), np.asarray(k0), np.asarray(r0),
        np.asarray(b0), np.asarray(k1), np.asarray(r1), np.asarray(b1),
        np.asarray(Wout), np.asarray(bout),
    )
    res = run_bass_kernel_spmd(
        nc, in_maps, core_ids=list(range(NC)), trace=_trace
    )
    kernel.last_result = res
    return res.results[0]["out"].astype(np.float32)

